# revision 8
# baseline (speedup 1.0000x reference)
"""Trainium2 Bass kernel for nn_FCBlock (dense MLP, 8-core data parallel).

Full (unsharded) inputs in, full output out. Internally: coords are sharded
along the batch axis across 8 NeuronCores, the (few-MB) weights are
replicated, and each core runs a fused 4-stage MLP entirely out of SBUF.

Math (per point, D=3, H=512, L=3):
  y = coords[:, 3:], u = coords[:, :3]
  z = relu(10p*(u@Wzu0 + bzu0 + (y*(u@Wyuu0+byuu0))@Wzyu[0]))
  u = relu(10pu*(u@Wu0 + bu0))
  for i in 1..3:
    zz = z * relu(10pzu*(u@Wzuu[i-1] + bzuu[i-1]))
    yy = y * (u@Wyuu[i-1] + byuu[i-1])
    t  = zz@Wzzu[i-1] + u@Wzu[i-1] + bzu[i-1] + yy@Wzyu[i]     (i<3)
    t  = zz@Wzzu_last + u@Wzu_last + bzu_last + yy@Wzyu_last   (i=3 -> output)
    z, u = relu(10p*t), relu(10pu*(u@Wu[i-1] + bu[i-1]))

All 10p/10pu/10pzu scales are folded into weights/biases on the host.

Perf-critical layout decisions (from trace analysis of the v0 kernel):
 * Every matmul streams K>=128 active rows so the PE HAM never sees "low
   activity" and re-throttles to 1.2 GHz: the D=3 input matmuls are fed a
   [128, BT] input built from 32 stacked copies of coords^T (weights /32),
   and the K=1 rank-1 (yy @ Wzyu) matmuls are eliminated entirely.
 * yy is produced BROADCAST across partitions by an M=128-replicated
   stationary weight (same PE cost as M=1), and the rank-1 t3 term is
   applied on DVE via scalar_tensor_tensor against PSUM.
 * y itself is broadcast by a matmul against the 32-replicated input
   (selecting the y rows), avoiding any gpsimd/DMA broadcast.
 * Weights are pre-transposed on the host so DMAs are contiguous, and
   ordered/split so layer-0/1 weights land before they are needed.
 * Epilogues are split across ACT (srl/unew relu+bias), DVE (all PSUM
   drains: rank-1 adds, yy, L0 u), GPSIMD (SBUF->SBUF relu+bias for
   z/znew) so no single engine gates the PE.
"""

import sys

sys.path.insert(0, "/opt/trn_rl_repo")

import numpy as np

import concourse.bacc as bacc
import concourse.mybir as mybir
from concourse.tile import TileContext
from concourse.bass_utils import run_bass_kernel_spmd

NCORES = 8
NFULL = 65536
NC_B = NFULL // NCORES  # 8192 points per core
H = 512
L = 3
D = 3
P = 128
HO = H // P  # 4 partition-tiles per hidden vector
BT = 1024  # batch tile (epilogue op width)
MW = 512   # matmul moving-slice width (PSUM bank limit)
NH = BT // MW
NT = NC_B // BT
REP = P // (D + 1)  # 32 stacked copies of [u0,u1,u2,y] rows

F32 = mybir.dt.float32
RELU = mybir.ActivationFunctionType.Relu
COPY = mybir.ActivationFunctionType.Copy
ADD = mybir.AluOpType.add
MULT = mybir.AluOpType.mult
MAX = mybir.AluOpType.max

_CACHE = {}


def _build(reps=1, mm_dt=mybir.dt.float16):
    nc = bacc.Bacc(trn_type="TRN2", name="fcblock")

    MDT = mm_dt
    ct_d = nc.dram_tensor("ct_d", [P, NC_B], MDT, kind="ExternalInput")
    yt_d = nc.dram_tensor("yt_d", [1, NC_B], F32, kind="ExternalInput")
    # w0: [128, 1280]: cols 0:512 z-block, 512:1024 u-block,
    #     1024:1152 yy0-block (M=128 replicated), 1152:1280 y-broadcast block
    w0_d = nc.dram_tensor("w0_d", [P, 2 * H + 2 * P], MDT, kind="ExternalInput")
    # wmain: [128, 9, 4, 512]; s = 4*li+j, j: 0=Wzuu 1=Wzzu 2=Wzu 3=Wu (li=0,1);
    #        s=8: Wzuu[2]
    wmain = nc.dram_tensor("wmain", [P, 9, HO, H], MDT, kind="ExternalInput")
    # wyb: [128, 2, 4, 128] — Wyuu[li] column replicated to M=128, per ko
    wyb_d = nc.dram_tensor("wyb_d", [P, 2, HO, P], MDT, kind="ExternalInput")
    # wl: [128, 4, 34] — col 0 Wyuu[2], col 32 Wzu_last (32-aligned so the
    # packed matmul output rows can be read by DVE), col 33 Wzzu_last
    wl_d = nc.dram_tensor("wl_d", [P, HO, 34], MDT, kind="ExternalInput")
    wzyu_d = nc.dram_tensor("wzyu_d", [P, L, HO], F32, kind="ExternalInput")
    bzuu_d = nc.dram_tensor("bzuu_d", [P, L, HO], F32, kind="ExternalInput")
    bu_d = nc.dram_tensor("bu_d", [P, L, HO], F32, kind="ExternalInput")
    bzu_d = nc.dram_tensor("bzu_d", [P, L, HO], F32, kind="ExternalInput")
    byuu_d = nc.dram_tensor("byuu_d", [P, L + 1], F32, kind="ExternalInput")
    sc_d = nc.dram_tensor("sc_d", [1, 2], F32, kind="ExternalInput")  # wzyul,bzul
    out = nc.dram_tensor("out", [1, NC_B], F32, kind="ExternalOutput")

    with TileContext(nc) as tc:
        with (
            tc.tile_pool(name="wpool", bufs=1) as wpool,
            tc.tile_pool(name="spool", bufs=2) as spool,
            tc.tile_pool(name="psum", bufs=4, space="PSUM") as psum,
        ):
            # --- resident weights/biases (small + first-needed first) -------
            w0 = wpool.tile([P, 2 * H + 2 * P], MDT, name="w0")
            nc.sync.dma_start(w0[:], w0_d[:, :])
            wzyu_t = wpool.tile([P, L, HO], F32, name="wzyu_t")
            nc.sync.dma_start(wzyu_t[:], wzyu_d[:, :, :])
            bzuu_t = wpool.tile([P, L, HO], F32, name="bzuu_t")
            nc.sync.dma_start(bzuu_t[:], bzuu_d[:, :, :])
            bu_t = wpool.tile([P, L, HO], F32, name="bu_t")
            nc.sync.dma_start(bu_t[:], bu_d[:, :, :])
            bzu_t = wpool.tile([P, L, HO], F32, name="bzu_t")
            nc.sync.dma_start(bzu_t[:], bzu_d[:, :, :])
            byuu_t = wpool.tile([P, L + 1], F32, name="byuu_t")
            nc.sync.dma_start(byuu_t[:], byuu_d[:, :])
            sc_t = wpool.tile([1, 2], F32, name="sc_t")
            nc.sync.dma_start(sc_t[:], sc_d[:, :])
            wyb = wpool.tile([P, 2, HO, P], MDT, name="wyb")
            nc.sync.dma_start(wyb[:], wyb_d[:, :, :, :])
            wl = wpool.tile([P, HO, 34], MDT, name="wl")
            nc.sync.dma_start(wl[:], wl_d[:, :, :])
            wm = wpool.tile([P, 9, HO, H], MDT, name="wm")
            nc.sync.dma_start(wm[:, 0:4], wmain[:, 0:4])  # layer-1 weights
            nc.sync.dma_start(wm[:, 4:8], wmain[:, 4:8])  # layer-2 weights
            nc.sync.dma_start(wm[:, 8:9], wmain[:, 8:9])  # layer-3 Wzuu

            def mm(pt, lhsT, rhs, start, stop):
                for h in range(NH):
                    nc.tensor.matmul(
                        pt[:, h * MW:(h + 1) * MW], lhsT,
                        rhs[:, h * MW:(h + 1) * MW], start=start, stop=stop)

            def emit_l0(t):
                """L0 for tile t: loads ct/yt, computes z, u, y_b, yy0_b."""
                ct = spool.tile([P, BT], MDT, name="ct", tag="ct", bufs=3)
                nc.sync.dma_start(ct[:], ct_d[:, t * BT:(t + 1) * BT])
                yt = spool.tile([1, BT], F32, name="yt", tag="yt", bufs=3)
                nc.sync.dma_start(yt[:], yt_d[:, t * BT:(t + 1) * BT])

                # q_ps = u@Wyuu0 broadcast to all 128 partitions
                q_ps = psum.tile([P, BT], F32, name="q_ps", tag="ps")
                mm(q_ps, w0[:, 2 * H:2 * H + P], ct, True, True)
                # yb_ps = y broadcast to all 128 partitions
                yb_ps = psum.tile([P, BT], F32, name="yb_ps", tag="ps")
                mm(yb_ps, w0[:, 2 * H + P:2 * H + 2 * P], ct, True, True)
                y_b = spool.tile([P, BT], MDT, name="y_b", tag="y_b", bufs=2)
                nc.scalar.activation(y_b[:], yb_ps[:], COPY)
                yy0 = spool.tile([P, BT], MDT, name="yy0", tag="yy0", bufs=2)
                # yy0 = (u@Wyuu0 + byuu0) * y  (q from PSUM, y from SBUF copy)
                nc.vector.scalar_tensor_tensor(
                    yy0[:], q_ps[:], byuu_t[:, 0:1], y_b[:], op0=ADD, op1=MULT)

                z = spool.tile([P, HO, BT], MDT, name="z", tag="z", bufs=3)
                u = spool.tile([P, HO, BT], MDT, name="u", tag="u", bufs=3)
                for mo in range(HO):
                    zp = psum.tile([P, BT], F32, name="zp", tag="ps")
                    mm(zp, w0[:, mo * P:(mo + 1) * P], ct, True, True)
                    tmp = spool.tile([P, BT], MDT, name="tmp", tag="tmp", bufs=4)
                    nc.vector.scalar_tensor_tensor(
                        tmp[:], yy0[:], wzyu_t[:, 0, mo:mo + 1], zp[:],
                        op0=MULT, op1=ADD)
                    nc.gpsimd.tensor_scalar(
                        z[:, mo, :], tmp[:], bzu_t[:, 0, mo:mo + 1], 0.0,
                        ADD, MAX)
                for mo in range(HO):
                    up = psum.tile([P, BT], F32, name="up", tag="ps")
                    mm(up, w0[:, H + mo * P:H + (mo + 1) * P], ct, True, True)
                    nc.vector.tensor_scalar(
                        u[:, mo, :], up[:], bu_t[:, 0, mo:mo + 1], 0.0,
                        ADD, MAX)
                return z, u, y_b, yt

            def emit_mid(li, z, u, y_b):
                """Layers 1..2 (li=0,1): returns (znew, unew)."""
                i = li + 1
                srl = spool.tile([P, HO, BT], MDT, name="srl", tag="srl", bufs=2)
                zz = spool.tile([P, HO, BT], MDT, name="zz", tag="zz", bufs=2)
                for mo in range(HO):
                    sp = psum.tile([P, BT], F32, name="sp", tag="ps")
                    for ko in range(HO):
                        mm(sp, wm[:, 4 * li + 0, ko, mo * P:(mo + 1) * P],
                           u[:, ko, :], ko == 0, ko == HO - 1)
                    nc.scalar.activation(
                        srl[:, mo, :], sp[:], RELU, bias=bzuu_t[:, li, mo:mo + 1])
                    nc.vector.tensor_mul(zz[:, mo, :], z[:, mo, :], srl[:, mo, :])

                # yy broadcast: M=128-replicated Wyuu[li] column
                pyp = psum.tile([P, BT], F32, name="pyp", tag="ps")
                for ko in range(HO):
                    mm(pyp, wyb[:, li, ko, :], u[:, ko, :], ko == 0, ko == HO - 1)
                yy_b = spool.tile([P, BT], MDT, name="yy_b", tag="yy_b", bufs=2)
                nc.vector.scalar_tensor_tensor(
                    yy_b[:], pyp[:], byuu_t[:, i:i + 1], y_b[:],
                    op0=ADD, op1=MULT)

                znew = spool.tile([P, HO, BT], MDT, name="znew", tag="z", bufs=3)
                unew = spool.tile([P, HO, BT], MDT, name="unew", tag="u", bufs=3)
                for mo in range(HO):
                    tp = psum.tile([P, BT], F32, name="tp", tag="ps")
                    for ko in range(HO):
                        mm(tp, wm[:, 4 * li + 1, ko, mo * P:(mo + 1) * P],
                           zz[:, ko, :], ko == 0, False)
                    for ko in range(HO):
                        mm(tp, wm[:, 4 * li + 2, ko, mo * P:(mo + 1) * P],
                           u[:, ko, :], False, ko == HO - 1)
                    tmp = spool.tile([P, BT], MDT, name="tmp", tag="tmp", bufs=4)
                    nc.vector.scalar_tensor_tensor(
                        tmp[:], yy_b[:], wzyu_t[:, i, mo:mo + 1], tp[:],
                        op0=MULT, op1=ADD)
                    nc.gpsimd.tensor_scalar(
                        znew[:, mo, :], tmp[:], bzu_t[:, i, mo:mo + 1], 0.0,
                        ADD, MAX)
                for mo in range(HO):
                    pn = psum.tile([P, BT], F32, name="pn", tag="ps")
                    for ko in range(HO):
                        mm(pn, wm[:, 4 * li + 3, ko, mo * P:(mo + 1) * P],
                           u[:, ko, :], ko == 0, ko == HO - 1)
                    nc.scalar.activation(
                        unew[:, mo, :], pn[:], RELU, bias=bu_t[:, i, mo:mo + 1])
                return znew, unew

            def emit_l3a(z, u):
                """L3 srl/zz (dense Wzuu[2] matmuls)."""
                srl = spool.tile([P, HO, BT], MDT, name="srl3", tag="srl", bufs=2)
                zz = spool.tile([P, HO, BT], MDT, name="zz3", tag="zz", bufs=2)
                for mo in range(HO):
                    sp = psum.tile([P, BT], F32, name="sp3", tag="ps")
                    for ko in range(HO):
                        mm(sp, wm[:, 8, ko, mo * P:(mo + 1) * P],
                           u[:, ko, :], ko == 0, ko == HO - 1)
                    nc.scalar.activation(
                        srl[:, mo, :], sp[:], RELU, bias=bzuu_t[:, 2, mo:mo + 1])
                    nc.vector.tensor_mul(zz[:, mo, :], z[:, mo, :], srl[:, mo, :])
                return zz

            def emit_l3b(t, zz, u, yt):
                """L3 final: packed (Wyuu[2], Wzu_last) + Wzzu_last + output."""
                pyu = psum.tile([33, BT], F32, name="pyu", tag="ps")
                for ko in range(HO):
                    mm(pyu, wl[:, ko, 0:33], u[:, ko, :], ko == 0, ko == HO - 1)
                pzz = psum.tile([1, BT], F32, name="pzz", tag="ps")
                for ko in range(HO):
                    mm(pzz, wl[:, ko, 33:34], zz[:, ko, :], ko == 0, ko == HO - 1)
                yy3 = spool.tile([1, BT], F32, name="yy3", tag="yy3", bufs=2)
                nc.vector.scalar_tensor_tensor(
                    yy3[:], pyu[0:1, :], byuu_t[0:1, 3:4], yt[:],
                    op0=ADD, op1=MULT)
                ta = spool.tile([1, BT], F32, name="ta", tag="ta", bufs=2)
                nc.vector.scalar_tensor_tensor(
                    ta[:], yy3[:], sc_t[0:1, 0:1], pzz[:], op0=MULT, op1=ADD)
                ot = spool.tile([1, BT], F32, name="ot", tag="ot", bufs=2)
                nc.vector.scalar_tensor_tensor(
                    ot[:], pyu[32:33, :], sc_t[0:1, 1:2], ta[:], op0=ADD, op1=ADD)
                nc.sync.dma_start(out[:, t * BT:(t + 1) * BT], ot[:])

            # --- software-pipelined tile loop --------------------------------
            import contextlib
            rep_ctx = (
                tc.For_i(0, reps, 1, hint_engines=tuple(nc.engines))
                if reps > 1 else contextlib.nullcontext()
            )
            with rep_ctx:
                state = emit_l0(0)
                for t in range(NT):
                    z, u, y_b, yt = state
                    z, u = emit_mid(0, z, u, y_b)
                    z, u = emit_mid(1, z, u, y_b)
                    zz3 = emit_l3a(z, u)
                    # L0 of the next tile goes here: its cheap 2-instr PSUM
                    # groups drain (DVE/ACT) while the PE fills L3b, and the
                    # PE never sits in a long low-K stretch.
                    if t + 1 < NT:
                        state = emit_l0(t + 1)
                    emit_l3b(t, zz3, u, yt)

    nc.compile()
    return nc


def _preprocess(inputs, np_wdt=np.float16):
    """Fold scales into weights, build the replicated/transposed tensors."""
    f = lambda k: np.asarray(inputs[k], dtype=np.float32)
    p10 = float(10.0 * np.float32(inputs["p"]))
    pu10 = float(10.0 * np.float32(inputs["pu"]))
    pzu10 = float(10.0 * np.float32(inputs["pzu"]))

    Wzuu, Wzzu, Wzu, Wu = f("Wzuu"), f("Wzzu"), f("Wzu"), f("Wu")
    # s-order: li-major (Wzuu,Wzzu,Wzu,Wu) for li=0,1 then Wzuu[2]
    ws = [Wzuu[0] * pzu10, Wzzu[0] * p10, Wzu[0] * p10, Wu[0] * pu10,
          Wzuu[1] * pzu10, Wzzu[1] * p10, Wzu[1] * p10, Wu[1] * pu10,
          Wzuu[2] * pzu10]
    wmain = np.stack(ws, axis=0)                        # [9, 512, 512]
    # [9, K=512, M=512] -> [p, s, ko, m]
    wmain_r = np.ascontiguousarray(
        wmain.reshape(9, HO, P, H).transpose(2, 0, 1, 3)).astype(np_wdt)

    # w0: rows = 32 copies of (u0,u1,u2,y); weights / 32
    Wzu0, Wu0, Wyuu0 = f("Wzu0"), f("Wu0"), f("Wyuu0")
    w0blk = np.zeros((D + 1, 2 * H + 2 * P), np.float32)
    w0blk[0:D, 0:H] = Wzu0 * p10
    w0blk[0:D, H:2 * H] = Wu0 * pu10
    w0blk[0:D, 2 * H:2 * H + P] = np.repeat(Wyuu0, P, axis=1)
    w0blk[D, 2 * H + P:2 * H + 2 * P] = 1.0
    w0 = np.tile(w0blk / REP, (REP, 1)).astype(np_wdt)  # [128, 1280]

    Wyuu = f("Wyuu")  # [L, H, 1]
    wyb = np.zeros((P, 2, HO, P), np.float32)
    for li in range(2):
        col = Wyuu[li, :, 0].reshape(HO, P)             # [ko, p]
        wyb[:, li, :, :] = col.T[:, :, None]            # replicate to M=128
    wyb = wyb.astype(np_wdt)

    wl = np.zeros((P, HO, 34), np.float32)
    wl[:, :, 0] = Wyuu[2, :, 0].reshape(HO, P).T
    wl[:, :, 32] = f("Wzu_last")[:, 0].reshape(HO, P).T
    wl[:, :, 33] = f("Wzzu_last")[:, 0].reshape(HO, P).T
    wl = wl.astype(np_wdt)

    Wzyu = f("Wzyu")  # [L, 1, H]
    wzyu_r = np.ascontiguousarray(
        (Wzyu[:, 0, :] * p10).reshape(L, HO, P).transpose(2, 0, 1)
    ).astype(np.float32)                                # [p, i, mo]

    def per_part(b, scale):                             # [L?, H] -> [p, l, mo]
        return np.ascontiguousarray(
            (b * scale).reshape(-1, HO, P).transpose(2, 0, 1)).astype(np.float32)

    bzuu = per_part(f("bzuu"), pzu10)                   # [128, 3, 4]
    bu = per_part(np.stack([f("bu0"), f("bu")[0], f("bu")[1]]), pu10)
    bzu = per_part(np.stack([f("bzu0"), f("bzu")[0], f("bzu")[1]]), p10)
    byuu = np.concatenate([f("byuu0"), f("byuu")[:, 0]])  # [4]
    byuu_b = np.tile(byuu[None, :], (P, 1)).astype(np.float32)
    sc = np.array([[f("Wzyu_last")[0, 0], f("bzu_last")[0]]], np.float32)

    return dict(
        w0_d=w0, wmain=wmain_r, wyb_d=wyb, wl_d=wl, wzyu_d=wzyu_r,
        bzuu_d=bzuu, bu_d=bu, bzu_d=bzu, byuu_d=byuu_b, sc_d=sc,
    )


MM_DTYPES = {
    "fp16": (mybir.dt.float16, np.float16),
    "bf16": (mybir.dt.bfloat16, None),  # ml_dtypes.bfloat16 filled lazily
}


def _run(inputs, trace=False, reps=1, mm="fp16", **kw):
    mdt, np_wdt = MM_DTYPES[mm]
    if np_wdt is None:
        import ml_dtypes
        np_wdt = ml_dtypes.bfloat16
    key = (reps, mm)
    if key not in _CACHE:
        _CACHE[key] = _build(reps, mm_dt=mdt)
    nc = _CACHE[key]
    shared = _preprocess(inputs, np_wdt)
    coords = np.asarray(inputs["coords"], dtype=np.float32)
    c4T = coords.T                                        # [4, NFULL]
    ctT = np.tile(c4T, (REP, 1)).astype(np_wdt)           # [128, NFULL]
    ytT = np.ascontiguousarray(c4T[D:D + 1, :])           # [1, NFULL]
    in_maps = [
        {**shared,
         "ct_d": np.ascontiguousarray(ctT[:, c * NC_B:(c + 1) * NC_B]),
         "yt_d": np.ascontiguousarray(ytT[:, c * NC_B:(c + 1) * NC_B])}
        for c in range(NCORES)
    ]
    res = run_bass_kernel_spmd(nc, in_maps, core_ids=list(range(NCORES)),
                               trace=trace, **kw)
    full = np.concatenate(
        [res.results[c]["out"].reshape(NC_B, 1) for c in range(NCORES)], axis=0)
    return full, res


def kernel(**inputs) -> np.ndarray:
    out, _ = _run(inputs)
    return out


# revision 13
# speedup vs baseline: 2.4122x; 2.4122x over previous
"""Trainium2 Bass kernel for nn_FCBlock (dense MLP, 8-core data parallel).

Full (unsharded) inputs in, full output out. Internally: coords are sharded
along the batch axis across 8 NeuronCores, the (few-MB) weights are
replicated, and each core runs a fused 4-stage MLP entirely out of SBUF.

Math (per point, D=3, H=512, L=3):
  y = coords[:, 3:], u = coords[:, :3]
  z = relu(10p*(u@Wzu0 + bzu0 + (y*(u@Wyuu0+byuu0))@Wzyu[0]))
  u = relu(10pu*(u@Wu0 + bu0))
  for i in 1..3:
    zz = z * relu(10pzu*(u@Wzuu[i-1] + bzuu[i-1]))
    yy = y * (u@Wyuu[i-1] + byuu[i-1])
    t  = zz@Wzzu[i-1] + u@Wzu[i-1] + bzu[i-1] + yy@Wzyu[i]     (i<3)
    t  = zz@Wzzu_last + u@Wzu_last + bzu_last + yy@Wzyu_last   (i=3 -> output)
    z, u = relu(10p*t), relu(10pu*(u@Wu[i-1] + bu[i-1]))

All 10p/10pu/10pzu scales are folded into weights/biases on the host.

Perf-critical layout decisions (from trace analysis of the v0 kernel):
 * Every matmul streams K>=128 active rows so the PE HAM never sees "low
   activity" and re-throttles to 1.2 GHz: the D=3 input matmuls are fed a
   [128, BT] input built from 32 stacked copies of coords^T (weights /32),
   and the K=1 rank-1 (yy @ Wzyu) matmuls are eliminated entirely.
 * yy is produced BROADCAST across partitions by an M=128-replicated
   stationary weight (same PE cost as M=1), and the rank-1 t3 term is
   applied on DVE via scalar_tensor_tensor against PSUM.
 * y itself is broadcast by a matmul against the 32-replicated input
   (selecting the y rows), avoiding any gpsimd/DMA broadcast.
 * Weights are pre-transposed on the host so DMAs are contiguous, and
   ordered/split so layer-0/1 weights land before they are needed.
 * Epilogues are split across ACT (srl/unew relu+bias), DVE (all PSUM
   drains: rank-1 adds, yy, L0 u), GPSIMD (SBUF->SBUF relu+bias for
   z/znew) so no single engine gates the PE.
"""

import sys

sys.path.insert(0, "/opt/trn_rl_repo")

import numpy as np

import concourse.bacc as bacc
import concourse.mybir as mybir
from concourse.tile import TileContext
from concourse.bass_utils import run_bass_kernel_spmd

NCORES = 8
NFULL = 65536
NC_B = NFULL // NCORES  # 8192 points per core
H = 512
L = 3
D = 3
P = 128
HO = H // P  # 4 partition-tiles per hidden vector
BT = 1024  # batch tile (epilogue op width)
MW = 512   # matmul moving-slice width (PSUM bank limit)
NH = BT // MW
NT = NC_B // BT
REP = P // (D + 1)  # 32 stacked copies of [u0,u1,u2,y] rows

F32 = mybir.dt.float32
RELU = mybir.ActivationFunctionType.Relu
COPY = mybir.ActivationFunctionType.Copy
ADD = mybir.AluOpType.add
MULT = mybir.AluOpType.mult
MAX = mybir.AluOpType.max

_CACHE = {}


def _build(reps=1, mm_dt=mybir.dt.float16):
    nc = bacc.Bacc(trn_type="TRN2", name="fcblock")

    MDT = mm_dt
    ct_d = nc.dram_tensor("ct_d", [P, NC_B], MDT, kind="ExternalInput")
    yt_d = nc.dram_tensor("yt_d", [1, NC_B], F32, kind="ExternalInput")
    # w0: [128, 1280]: cols 0:512 z-block, 512:1024 u-block,
    #     1024:1152 yy0-block (M=128 replicated), 1152:1280 y-broadcast block
    w0_d = nc.dram_tensor("w0_d", [P, 2 * H + 2 * P], MDT, kind="ExternalInput")
    # wmain: [128, 9, 4, 512]; s = 4*li+j, j: 0=Wzuu 1=Wzzu 2=Wzu 3=Wu (li=0,1);
    #        s=8: Wzuu[2]
    wmain = nc.dram_tensor("wmain", [P, 9, HO, H], MDT, kind="ExternalInput")
    # wyb: [128, 2, 4, 128] — Wyuu[li] column replicated to M=128, per ko
    wyb_d = nc.dram_tensor("wyb_d", [P, 2, HO, P], MDT, kind="ExternalInput")
    # wl: [128, 4, 34] — col 0 Wyuu[2], col 32 Wzu_last (32-aligned so the
    # packed matmul output rows can be read by DVE), col 33 Wzzu_last
    wl_d = nc.dram_tensor("wl_d", [P, HO, 34], MDT, kind="ExternalInput")
    wzyu_d = nc.dram_tensor("wzyu_d", [P, L, HO], F32, kind="ExternalInput")
    bzuu_d = nc.dram_tensor("bzuu_d", [P, L, HO], F32, kind="ExternalInput")
    bu_d = nc.dram_tensor("bu_d", [P, L, HO], F32, kind="ExternalInput")
    bzu_d = nc.dram_tensor("bzu_d", [P, L, HO], F32, kind="ExternalInput")
    byuu_d = nc.dram_tensor("byuu_d", [P, L + 1], F32, kind="ExternalInput")
    sc_d = nc.dram_tensor("sc_d", [1, 2], F32, kind="ExternalInput")  # wzyul,bzul
    out = nc.dram_tensor("out", [1, NC_B], F32, kind="ExternalOutput")

    with TileContext(nc) as tc:
        with (
            tc.tile_pool(name="wpool", bufs=1) as wpool,
            tc.tile_pool(name="spool", bufs=2) as spool,
            tc.tile_pool(name="psum", bufs=4, space="PSUM") as psum,
        ):
            # --- tile-0 inputs first: the DMA rings drain in issue order, so
            # these must not queue behind the ~5MB of weights ----------------
            ct0 = spool.tile([P, BT], MDT, name="ct", tag="ct", bufs=3)
            nc.sync.dma_start(ct0[:], ct_d[:, 0:BT])
            yt0 = spool.tile([1, BT], F32, name="yt", tag="yt", bufs=3)
            nc.sync.dma_start(yt0[:], yt_d[:, 0:BT])

            # --- resident weights/biases (small + first-needed first) -------
            w0 = wpool.tile([P, 2 * H + 2 * P], MDT, name="w0")
            nc.sync.dma_start(w0[:], w0_d[:, :])
            wzyu_t = wpool.tile([P, L, HO], F32, name="wzyu_t")
            nc.sync.dma_start(wzyu_t[:], wzyu_d[:, :, :])
            bzuu_t = wpool.tile([P, L, HO], F32, name="bzuu_t")
            nc.sync.dma_start(bzuu_t[:], bzuu_d[:, :, :])
            bu_t = wpool.tile([P, L, HO], F32, name="bu_t")
            nc.sync.dma_start(bu_t[:], bu_d[:, :, :])
            bzu_t = wpool.tile([P, L, HO], F32, name="bzu_t")
            nc.sync.dma_start(bzu_t[:], bzu_d[:, :, :])
            byuu_t = wpool.tile([P, L + 1], F32, name="byuu_t")
            nc.sync.dma_start(byuu_t[:], byuu_d[:, :])
            sc_t = wpool.tile([1, 2], F32, name="sc_t")
            nc.sync.dma_start(sc_t[:], sc_d[:, :])
            wyb = wpool.tile([P, 2, HO, P], MDT, name="wyb")
            nc.sync.dma_start(wyb[:], wyb_d[:, :, :, :])
            wl = wpool.tile([P, HO, 34], MDT, name="wl")
            nc.sync.dma_start(wl[:], wl_d[:, :, :])
            wm = wpool.tile([P, 9, HO, H], MDT, name="wm")
            nc.sync.dma_start(wm[:, 0:4], wmain[:, 0:4])  # layer-1 weights
            nc.sync.dma_start(wm[:, 4:8], wmain[:, 4:8])  # layer-2 weights
            nc.sync.dma_start(wm[:, 8:9], wmain[:, 8:9])  # layer-3 Wzuu

            def mm(pt, lhsT, rhs, start, stop):
                for h in range(NH):
                    nc.tensor.matmul(
                        pt[:, h * MW:(h + 1) * MW], lhsT,
                        rhs[:, h * MW:(h + 1) * MW], start=start, stop=stop)

            def emit_l0(t, ct=None, yt=None):
                """L0 for tile t: loads ct/yt, computes z, u, y_b, yy0_b."""
                if ct is None:
                    ct = spool.tile([P, BT], MDT, name="ct", tag="ct", bufs=3)
                    nc.sync.dma_start(ct[:], ct_d[:, t * BT:(t + 1) * BT])
                    yt = spool.tile([1, BT], F32, name="yt", tag="yt", bufs=3)
                    nc.sync.dma_start(yt[:], yt_d[:, t * BT:(t + 1) * BT])

                # q_ps = u@Wyuu0 broadcast to all 128 partitions
                q_ps = psum.tile([P, BT], F32, name="q_ps", tag="ps")
                mm(q_ps, w0[:, 2 * H:2 * H + P], ct, True, True)
                # yb_ps = y broadcast to all 128 partitions
                yb_ps = psum.tile([P, BT], F32, name="yb_ps", tag="ps")
                mm(yb_ps, w0[:, 2 * H + P:2 * H + 2 * P], ct, True, True)
                y_b = spool.tile([P, BT], MDT, name="y_b", tag="y_b", bufs=2)
                nc.scalar.activation(y_b[:], yb_ps[:], COPY)
                yy0 = spool.tile([P, BT], MDT, name="yy0", tag="yy0", bufs=2)
                # yy0 = (u@Wyuu0 + byuu0) * y  (q from PSUM, y from SBUF copy)
                nc.vector.scalar_tensor_tensor(
                    yy0[:], q_ps[:], byuu_t[:, 0:1], y_b[:], op0=ADD, op1=MULT)

                z = spool.tile([P, HO, BT], MDT, name="z", tag="z", bufs=3)
                u = spool.tile([P, HO, BT], MDT, name="u", tag="u", bufs=3)
                for mo in range(HO):
                    zp = psum.tile([P, BT], F32, name="zp", tag="ps")
                    mm(zp, w0[:, mo * P:(mo + 1) * P], ct, True, True)
                    tmp = spool.tile([P, BT], MDT, name="tmp", tag="tmp", bufs=4)
                    nc.vector.scalar_tensor_tensor(
                        tmp[:], yy0[:], wzyu_t[:, 0, mo:mo + 1], zp[:],
                        op0=MULT, op1=ADD)
                    nc.scalar.activation(
                        z[:, mo, :], tmp[:], RELU, bias=bzu_t[:, 0, mo:mo + 1])
                for mo in range(HO):
                    up = psum.tile([P, BT], F32, name="up", tag="ps")
                    mm(up, w0[:, H + mo * P:H + (mo + 1) * P], ct, True, True)
                    nc.scalar.activation(
                        u[:, mo, :], up[:], RELU, bias=bu_t[:, 0, mo:mo + 1])
                return z, u, y_b, yt

            def emit_mid(li, z, u, y_b):
                """Layers 1..2 (li=0,1): returns (znew, unew)."""
                i = li + 1
                srl = spool.tile([P, HO, BT], MDT, name="srl", tag="srl", bufs=2)
                zz = spool.tile([P, HO, BT], MDT, name="zz", tag="zz", bufs=2)
                for mo in range(HO):
                    sp = psum.tile([P, BT], F32, name="sp", tag="ps")
                    for ko in range(HO):
                        mm(sp, wm[:, 4 * li + 0, ko, mo * P:(mo + 1) * P],
                           u[:, ko, :], ko == 0, ko == HO - 1)
                    nc.scalar.activation(
                        srl[:, mo, :], sp[:], RELU, bias=bzuu_t[:, li, mo:mo + 1])
                    nc.vector.tensor_mul(zz[:, mo, :], z[:, mo, :], srl[:, mo, :])

                # yy broadcast: M=128-replicated Wyuu[li] column
                pyp = psum.tile([P, BT], F32, name="pyp", tag="ps")
                for ko in range(HO):
                    mm(pyp, wyb[:, li, ko, :], u[:, ko, :], ko == 0, ko == HO - 1)
                yy_b = spool.tile([P, BT], MDT, name="yy_b", tag="yy_b", bufs=2)
                nc.vector.scalar_tensor_tensor(
                    yy_b[:], pyp[:], byuu_t[:, i:i + 1], y_b[:],
                    op0=ADD, op1=MULT)

                znew = spool.tile([P, HO, BT], MDT, name="znew", tag="z", bufs=3)
                unew = spool.tile([P, HO, BT], MDT, name="unew", tag="u", bufs=3)
                for mo in range(HO):
                    tp = psum.tile([P, BT], F32, name="tp", tag="ps")
                    for ko in range(HO):
                        mm(tp, wm[:, 4 * li + 1, ko, mo * P:(mo + 1) * P],
                           zz[:, ko, :], ko == 0, False)
                    for ko in range(HO):
                        mm(tp, wm[:, 4 * li + 2, ko, mo * P:(mo + 1) * P],
                           u[:, ko, :], False, ko == HO - 1)
                    tmp = spool.tile([P, BT], MDT, name="tmp", tag="tmp", bufs=4)
                    nc.vector.scalar_tensor_tensor(
                        tmp[:], yy_b[:], wzyu_t[:, i, mo:mo + 1], tp[:],
                        op0=MULT, op1=ADD)
                    nc.scalar.activation(
                        znew[:, mo, :], tmp[:], RELU,
                        bias=bzu_t[:, i, mo:mo + 1])
                for mo in range(HO):
                    pn = psum.tile([P, BT], F32, name="pn", tag="ps")
                    for ko in range(HO):
                        mm(pn, wm[:, 4 * li + 3, ko, mo * P:(mo + 1) * P],
                           u[:, ko, :], ko == 0, ko == HO - 1)
                    nc.scalar.activation(
                        unew[:, mo, :], pn[:], RELU, bias=bu_t[:, i, mo:mo + 1])
                return znew, unew

            def emit_l3a(z, u):
                """L3 srl/zz (dense Wzuu[2] matmuls)."""
                srl = spool.tile([P, HO, BT], MDT, name="srl3", tag="srl", bufs=2)
                zz = spool.tile([P, HO, BT], MDT, name="zz3", tag="zz", bufs=2)
                for mo in range(HO):
                    sp = psum.tile([P, BT], F32, name="sp3", tag="ps")
                    for ko in range(HO):
                        mm(sp, wm[:, 8, ko, mo * P:(mo + 1) * P],
                           u[:, ko, :], ko == 0, ko == HO - 1)
                    nc.scalar.activation(
                        srl[:, mo, :], sp[:], RELU, bias=bzuu_t[:, 2, mo:mo + 1])
                    nc.vector.tensor_mul(zz[:, mo, :], z[:, mo, :], srl[:, mo, :])
                return zz

            def emit_l3b(t, zz, u, yt):
                """L3 final: packed (Wyuu[2], Wzu_last) + Wzzu_last + output."""
                pyu = psum.tile([33, BT], F32, name="pyu", tag="ps")
                for ko in range(HO):
                    mm(pyu, wl[:, ko, 0:33], u[:, ko, :], ko == 0, ko == HO - 1)
                pzz = psum.tile([1, BT], F32, name="pzz", tag="ps")
                for ko in range(HO):
                    mm(pzz, wl[:, ko, 33:34], zz[:, ko, :], ko == 0, ko == HO - 1)
                yy3 = spool.tile([1, BT], F32, name="yy3", tag="yy3", bufs=2)
                nc.vector.scalar_tensor_tensor(
                    yy3[:], pyu[0:1, :], byuu_t[0:1, 3:4], yt[:],
                    op0=ADD, op1=MULT)
                ta = spool.tile([1, BT], F32, name="ta", tag="ta", bufs=2)
                nc.vector.scalar_tensor_tensor(
                    ta[:], yy3[:], sc_t[0:1, 0:1], pzz[:], op0=MULT, op1=ADD)
                ot = spool.tile([1, BT], F32, name="ot", tag="ot", bufs=2)
                nc.vector.scalar_tensor_tensor(
                    ot[:], pyu[32:33, :], sc_t[0:1, 1:2], ta[:], op0=ADD, op1=ADD)
                nc.sync.dma_start(out[:, t * BT:(t + 1) * BT], ot[:])

            # --- software-pipelined tile loop --------------------------------
            import contextlib
            rep_ctx = (
                tc.For_i(0, reps, 1, hint_engines=tuple(nc.engines))
                if reps > 1 else contextlib.nullcontext()
            )
            with rep_ctx:
                state = emit_l0(0, ct0, yt0)
                for t in range(NT):
                    z, u, y_b, yt = state
                    z, u = emit_mid(0, z, u, y_b)
                    z, u = emit_mid(1, z, u, y_b)
                    zz3 = emit_l3a(z, u)
                    # L0 of the next tile goes here: its cheap 2-instr PSUM
                    # groups drain (DVE/ACT) while the PE fills L3b, and the
                    # PE never sits in a long low-K stretch.
                    if t + 1 < NT:
                        state = emit_l0(t + 1)
                    emit_l3b(t, zz3, u, yt)

    nc.compile()
    return nc


def _preprocess(inputs, np_wdt=np.float16):
    """Fold scales into weights, build the replicated/transposed tensors."""
    f = lambda k: np.asarray(inputs[k], dtype=np.float32)
    p10 = float(10.0 * np.float32(inputs["p"]))
    pu10 = float(10.0 * np.float32(inputs["pu"]))
    pzu10 = float(10.0 * np.float32(inputs["pzu"]))

    Wzuu, Wzzu, Wzu, Wu = f("Wzuu"), f("Wzzu"), f("Wzu"), f("Wu")
    # s-order: li-major (Wzuu,Wzzu,Wzu,Wu) for li=0,1 then Wzuu[2]
    ws = [Wzuu[0] * pzu10, Wzzu[0] * p10, Wzu[0] * p10, Wu[0] * pu10,
          Wzuu[1] * pzu10, Wzzu[1] * p10, Wzu[1] * p10, Wu[1] * pu10,
          Wzuu[2] * pzu10]
    wmain = np.stack(ws, axis=0)                        # [9, 512, 512]
    # [9, K=512, M=512] -> [p, s, ko, m]
    wmain_r = np.ascontiguousarray(
        wmain.reshape(9, HO, P, H).transpose(2, 0, 1, 3)).astype(np_wdt)

    # w0: rows = 32 copies of (u0,u1,u2,y); weights / 32
    Wzu0, Wu0, Wyuu0 = f("Wzu0"), f("Wu0"), f("Wyuu0")
    w0blk = np.zeros((D + 1, 2 * H + 2 * P), np.float32)
    w0blk[0:D, 0:H] = Wzu0 * p10
    w0blk[0:D, H:2 * H] = Wu0 * pu10
    w0blk[0:D, 2 * H:2 * H + P] = np.repeat(Wyuu0, P, axis=1)
    w0blk[D, 2 * H + P:2 * H + 2 * P] = 1.0
    w0 = np.tile(w0blk / REP, (REP, 1)).astype(np_wdt)  # [128, 1280]

    Wyuu = f("Wyuu")  # [L, H, 1]
    wyb = np.zeros((P, 2, HO, P), np.float32)
    for li in range(2):
        col = Wyuu[li, :, 0].reshape(HO, P)             # [ko, p]
        wyb[:, li, :, :] = col.T[:, :, None]            # replicate to M=128
    wyb = wyb.astype(np_wdt)

    wl = np.zeros((P, HO, 34), np.float32)
    wl[:, :, 0] = Wyuu[2, :, 0].reshape(HO, P).T
    wl[:, :, 32] = f("Wzu_last")[:, 0].reshape(HO, P).T
    wl[:, :, 33] = f("Wzzu_last")[:, 0].reshape(HO, P).T
    wl = wl.astype(np_wdt)

    Wzyu = f("Wzyu")  # [L, 1, H]
    wzyu_r = np.ascontiguousarray(
        (Wzyu[:, 0, :] * p10).reshape(L, HO, P).transpose(2, 0, 1)
    ).astype(np.float32)                                # [p, i, mo]

    def per_part(b, scale):                             # [L?, H] -> [p, l, mo]
        return np.ascontiguousarray(
            (b * scale).reshape(-1, HO, P).transpose(2, 0, 1)).astype(np.float32)

    bzuu = per_part(f("bzuu"), pzu10)                   # [128, 3, 4]
    bu = per_part(np.stack([f("bu0"), f("bu")[0], f("bu")[1]]), pu10)
    bzu = per_part(np.stack([f("bzu0"), f("bzu")[0], f("bzu")[1]]), p10)
    byuu = np.concatenate([f("byuu0"), f("byuu")[:, 0]])  # [4]
    byuu_b = np.tile(byuu[None, :], (P, 1)).astype(np.float32)
    sc = np.array([[f("Wzyu_last")[0, 0], f("bzu_last")[0]]], np.float32)

    return dict(
        w0_d=w0, wmain=wmain_r, wyb_d=wyb, wl_d=wl, wzyu_d=wzyu_r,
        bzuu_d=bzuu, bu_d=bu, bzu_d=bzu, byuu_d=byuu_b, sc_d=sc,
    )


MM_DTYPES = {
    "fp16": (mybir.dt.float16, np.float16),
    "bf16": (mybir.dt.bfloat16, None),  # ml_dtypes.bfloat16 filled lazily
}


def _run(inputs, trace=False, reps=1, mm="fp16", **kw):
    mdt, np_wdt = MM_DTYPES[mm]
    if np_wdt is None:
        import ml_dtypes
        np_wdt = ml_dtypes.bfloat16
    key = (reps, mm)
    if key not in _CACHE:
        _CACHE[key] = _build(reps, mm_dt=mdt)
    nc = _CACHE[key]
    shared = _preprocess(inputs, np_wdt)
    coords = np.asarray(inputs["coords"], dtype=np.float32)
    c4T = coords.T                                        # [4, NFULL]
    ctT = np.tile(c4T, (REP, 1)).astype(np_wdt)           # [128, NFULL]
    ytT = np.ascontiguousarray(c4T[D:D + 1, :])           # [1, NFULL]
    in_maps = [
        {**shared,
         "ct_d": np.ascontiguousarray(ctT[:, c * NC_B:(c + 1) * NC_B]),
         "yt_d": np.ascontiguousarray(ytT[:, c * NC_B:(c + 1) * NC_B])}
        for c in range(NCORES)
    ]
    res = run_bass_kernel_spmd(nc, in_maps, core_ids=list(range(NCORES)),
                               trace=trace, **kw)
    full = np.concatenate(
        [res.results[c]["out"].reshape(NC_B, 1) for c in range(NCORES)], axis=0)
    return full, res


def kernel(**inputs) -> np.ndarray:
    out, _ = _run(inputs)
    return out


# revision 19
# speedup vs baseline: 2.5038x; 1.0380x over previous
"""Trainium2 Bass kernel for nn_FCBlock (dense MLP, 8-core data parallel).

Full (unsharded) inputs in, full output out. Internally: coords are sharded
along the batch axis across 8 NeuronCores, the (few-MB) weights are
replicated, and each core runs a fused 4-stage MLP entirely out of SBUF.

Math (per point, D=3, H=512, L=3):
  y = coords[:, 3:], u = coords[:, :3]
  z = relu(10p*(u@Wzu0 + bzu0 + (y*(u@Wyuu0+byuu0))@Wzyu[0]))
  u = relu(10pu*(u@Wu0 + bu0))
  for i in 1..3:
    zz = z * relu(10pzu*(u@Wzuu[i-1] + bzuu[i-1]))
    yy = y * (u@Wyuu[i-1] + byuu[i-1])
    t  = zz@Wzzu[i-1] + u@Wzu[i-1] + bzu[i-1] + yy@Wzyu[i]     (i<3)
    t  = zz@Wzzu_last + u@Wzu_last + bzu_last + yy@Wzyu_last   (i=3 -> output)
    z, u = relu(10p*t), relu(10pu*(u@Wu[i-1] + bu[i-1]))

All 10p/10pu/10pzu scales are folded into weights/biases on the host.

Perf-critical layout decisions (from trace analysis of the v0 kernel):
 * Every matmul streams K>=128 active rows so the PE HAM never sees "low
   activity" and re-throttles to 1.2 GHz: the D=3 input matmuls are fed a
   [128, BT] input built from 32 stacked copies of coords^T (weights /32),
   and the K=1 rank-1 (yy @ Wzyu) matmuls are eliminated entirely.
 * yy is produced BROADCAST across partitions by an M=128-replicated
   stationary weight (same PE cost as M=1), and the rank-1 t3 term is
   applied on DVE via scalar_tensor_tensor against PSUM.
 * y itself is broadcast by a matmul against the 32-replicated input
   (selecting the y rows), avoiding any gpsimd/DMA broadcast.
 * Weights are pre-transposed on the host so DMAs are contiguous, and
   ordered/split so layer-0/1 weights land before they are needed.
 * Epilogues are split across ACT (srl/unew relu+bias), DVE (all PSUM
   drains: rank-1 adds, yy, L0 u), GPSIMD (SBUF->SBUF relu+bias for
   z/znew) so no single engine gates the PE.
"""

import sys

sys.path.insert(0, "/opt/trn_rl_repo")

import numpy as np

import concourse.bacc as bacc
import concourse.mybir as mybir
from concourse.tile import TileContext
from concourse.bass_utils import run_bass_kernel_spmd

NCORES = 8
NFULL = 65536
NC_B = NFULL // NCORES  # 8192 points per core
H = 512
L = 3
D = 3
P = 128
HO = H // P  # 4 partition-tiles per hidden vector
BT = 1024  # batch tile (epilogue op width)
MW = 512   # matmul moving-slice width (PSUM bank limit)
NH = BT // MW
NT = NC_B // BT
REP = P // (D + 1)  # 32 stacked copies of [u0,u1,u2,y] rows

F32 = mybir.dt.float32
RELU = mybir.ActivationFunctionType.Relu
COPY = mybir.ActivationFunctionType.Copy
ADD = mybir.AluOpType.add
MULT = mybir.AluOpType.mult
MAX = mybir.AluOpType.max

_CACHE = {}


def _build(reps=1, mm_dt=mybir.dt.float16):
    nc = bacc.Bacc(trn_type="TRN2", name="fcblock")

    MDT = mm_dt
    ct_d = nc.dram_tensor("ct_d", [P, NC_B], MDT, kind="ExternalInput")
    yt_d = nc.dram_tensor("yt_d", [1, NC_B], F32, kind="ExternalInput")
    # w0: [128, 1280]: cols 0:512 z-block, 512:1024 u-block,
    #     1024:1152 yy0-block (M=128 replicated), 1152:1280 y-broadcast block
    w0_d = nc.dram_tensor("w0_d", [P, 2 * H + 2 * P], MDT, kind="ExternalInput")
    # wmain: [128, 9, 4, 512]; s = 4*li+j, j: 0=Wzuu 1=Wzzu 2=Wzu 3=Wu (li=0,1);
    #        s=8: Wzuu[2]
    wmain = nc.dram_tensor("wmain", [P, 9, HO, H], MDT, kind="ExternalInput")
    # wyb: [128, 2, 4, 128] — Wyuu[li] column replicated to M=128, per ko
    wyb_d = nc.dram_tensor("wyb_d", [P, 2, HO, P], MDT, kind="ExternalInput")
    # wl: [128, 4, 2, 128] — two M=128-padded stationary blocks (full-width so
    # the HAM never sees a low-activity stretch): block 0 has Wyuu[2] in col 0
    # and Wzu_last in col 32 (32-aligned for DVE reads), block 1 has
    # Wzzu_last in col 0; all other columns are zero.
    wl_d = nc.dram_tensor("wl_d", [P, HO, 2, P], MDT, kind="ExternalInput")
    wzyu_d = nc.dram_tensor("wzyu_d", [P, L, HO], F32, kind="ExternalInput")
    bzuu_d = nc.dram_tensor("bzuu_d", [P, L, HO], F32, kind="ExternalInput")
    bu_d = nc.dram_tensor("bu_d", [P, L, HO], F32, kind="ExternalInput")
    bzu_d = nc.dram_tensor("bzu_d", [P, L, HO], F32, kind="ExternalInput")
    byuu_d = nc.dram_tensor("byuu_d", [P, L + 1], F32, kind="ExternalInput")
    sc_d = nc.dram_tensor("sc_d", [1, 2], F32, kind="ExternalInput")  # wzyul,bzul
    out = nc.dram_tensor("out", [1, NC_B], F32, kind="ExternalOutput")

    with TileContext(nc) as tc:
        with (
            tc.tile_pool(name="wpool", bufs=1) as wpool,
            tc.tile_pool(name="spool", bufs=2) as spool,
            tc.tile_pool(name="psum", bufs=4, space="PSUM") as psum,
        ):
            # --- tile-0 inputs first: the DMA rings drain in issue order, so
            # these must not queue behind the ~5MB of weights ----------------
            ct0 = spool.tile([P, BT], MDT, name="ct", tag="ct", bufs=3)
            nc.sync.dma_start(ct0[:], ct_d[:, 0:BT])
            yt0 = spool.tile([1, BT], F32, name="yt", tag="yt", bufs=3)
            nc.sync.dma_start(yt0[:], yt_d[:, 0:BT])

            # --- resident weights/biases (small + first-needed first) -------
            w0 = wpool.tile([P, 2 * H + 2 * P], MDT, name="w0")
            nc.sync.dma_start(w0[:], w0_d[:, :])
            wzyu_t = wpool.tile([P, L, HO], F32, name="wzyu_t")
            nc.sync.dma_start(wzyu_t[:], wzyu_d[:, :, :])
            bzuu_t = wpool.tile([P, L, HO], F32, name="bzuu_t")
            nc.sync.dma_start(bzuu_t[:], bzuu_d[:, :, :])
            bu_t = wpool.tile([P, L, HO], F32, name="bu_t")
            nc.sync.dma_start(bu_t[:], bu_d[:, :, :])
            bzu_t = wpool.tile([P, L, HO], F32, name="bzu_t")
            nc.sync.dma_start(bzu_t[:], bzu_d[:, :, :])
            byuu_t = wpool.tile([P, L + 1], F32, name="byuu_t")
            nc.sync.dma_start(byuu_t[:], byuu_d[:, :])
            sc_t = wpool.tile([1, 2], F32, name="sc_t")
            nc.sync.dma_start(sc_t[:], sc_d[:, :])
            wyb = wpool.tile([P, 2, HO, P], MDT, name="wyb")
            nc.sync.dma_start(wyb[:], wyb_d[:, :, :, :])
            wl = wpool.tile([P, HO, 2, P], MDT, name="wl")
            nc.sync.dma_start(wl[:], wl_d[:, :, :])
            wm = wpool.tile([P, 9, HO, H], MDT, name="wm")
            nc.sync.dma_start(wm[:, 0:4], wmain[:, 0:4])  # layer-1 weights
            nc.sync.dma_start(wm[:, 4:8], wmain[:, 4:8])  # layer-2 weights
            nc.sync.dma_start(wm[:, 8:9], wmain[:, 8:9])  # layer-3 Wzuu

            def mm(pt, lhsT, rhs, start, stop):
                for h in range(NH):
                    nc.tensor.matmul(
                        pt[:, h * MW:(h + 1) * MW], lhsT,
                        rhs[:, h * MW:(h + 1) * MW], start=start, stop=stop)

            def emit_l0(t, ct=None, yt=None):
                """L0 for tile t: loads ct/yt, computes z, u, y_b, yy0_b."""
                if ct is None:
                    ct = spool.tile([P, BT], MDT, name="ct", tag="ct", bufs=3)
                    nc.sync.dma_start(ct[:], ct_d[:, t * BT:(t + 1) * BT])
                    yt = spool.tile([1, BT], F32, name="yt", tag="yt", bufs=3)
                    nc.sync.dma_start(yt[:], yt_d[:, t * BT:(t + 1) * BT])

                # q_ps = u@Wyuu0 broadcast to all 128 partitions
                q_ps = psum.tile([P, BT], F32, name="q_ps", tag="ps")
                mm(q_ps, w0[:, 2 * H:2 * H + P], ct, True, True)
                # yb_ps = y broadcast to all 128 partitions
                yb_ps = psum.tile([P, BT], F32, name="yb_ps", tag="ps")
                mm(yb_ps, w0[:, 2 * H + P:2 * H + 2 * P], ct, True, True)
                y_b = spool.tile([P, BT], MDT, name="y_b", tag="y_b", bufs=2)
                nc.scalar.activation(y_b[:], yb_ps[:], COPY)
                yy0 = spool.tile([P, BT], MDT, name="yy0", tag="yy0", bufs=2)
                # yy0 = (u@Wyuu0 + byuu0) * y  (q from PSUM, y from SBUF copy)
                nc.vector.scalar_tensor_tensor(
                    yy0[:], q_ps[:], byuu_t[:, 0:1], y_b[:], op0=ADD, op1=MULT)

                z = spool.tile([P, HO, BT], MDT, name="z", tag="z", bufs=3)
                u = spool.tile([P, HO, BT], MDT, name="u", tag="u", bufs=3)
                # u-groups first: their epilogues gate the next layer's srl
                # matmuls, so give ACT a head start while the PE fills z.
                for mo in range(HO):
                    up = psum.tile([P, BT], F32, name="up", tag="ps")
                    mm(up, w0[:, H + mo * P:H + (mo + 1) * P], ct, True, True)
                    nc.scalar.activation(
                        u[:, mo, :], up[:], RELU, bias=bu_t[:, 0, mo:mo + 1])
                for mo in range(HO):
                    zp = psum.tile([P, BT], F32, name="zp", tag="ps")
                    mm(zp, w0[:, mo * P:(mo + 1) * P], ct, True, True)
                    tmp = spool.tile([P, BT], MDT, name="tmp", tag="tmp", bufs=4)
                    nc.vector.scalar_tensor_tensor(
                        tmp[:], yy0[:], wzyu_t[:, 0, mo:mo + 1], zp[:],
                        op0=MULT, op1=ADD)
                    nc.scalar.activation(
                        z[:, mo, :], tmp[:], RELU, bias=bzu_t[:, 0, mo:mo + 1])
                return z, u, y_b, yt

            def emit_mid(li, z, u, y_b):
                """Layers 1..2 (li=0,1): returns (znew, unew)."""
                i = li + 1
                srl = spool.tile([P, HO, BT], MDT, name="srl", tag="srl", bufs=2)
                zz = spool.tile([P, HO, BT], MDT, name="zz", tag="zz", bufs=2)
                for mo in range(HO):
                    sp = psum.tile([P, BT], F32, name="sp", tag="ps")
                    for ko in range(HO):
                        mm(sp, wm[:, 4 * li + 0, ko, mo * P:(mo + 1) * P],
                           u[:, ko, :], ko == 0, ko == HO - 1)
                    nc.scalar.activation(
                        srl[:, mo, :], sp[:], RELU, bias=bzuu_t[:, li, mo:mo + 1])
                    nc.vector.tensor_mul(zz[:, mo, :], z[:, mo, :], srl[:, mo, :])

                # yy broadcast: M=128-replicated Wyuu[li] column
                pyp = psum.tile([P, BT], F32, name="pyp", tag="ps")
                for ko in range(HO):
                    mm(pyp, wyb[:, li, ko, :], u[:, ko, :], ko == 0, ko == HO - 1)
                yy_b = spool.tile([P, BT], MDT, name="yy_b", tag="yy_b", bufs=2)
                nc.vector.scalar_tensor_tensor(
                    yy_b[:], pyp[:], byuu_t[:, i:i + 1], y_b[:],
                    op0=ADD, op1=MULT)

                znew = spool.tile([P, HO, BT], MDT, name="znew", tag="z", bufs=3)
                unew = spool.tile([P, HO, BT], MDT, name="unew", tag="u", bufs=3)
                for mo in range(HO):
                    tp = psum.tile([P, BT], F32, name="tp", tag="ps")
                    for ko in range(HO):
                        mm(tp, wm[:, 4 * li + 1, ko, mo * P:(mo + 1) * P],
                           zz[:, ko, :], ko == 0, False)
                    for ko in range(HO):
                        mm(tp, wm[:, 4 * li + 2, ko, mo * P:(mo + 1) * P],
                           u[:, ko, :], False, ko == HO - 1)
                    tmp = spool.tile([P, BT], MDT, name="tmp", tag="tmp", bufs=4)
                    nc.vector.scalar_tensor_tensor(
                        tmp[:], yy_b[:], wzyu_t[:, i, mo:mo + 1], tp[:],
                        op0=MULT, op1=ADD)
                    nc.scalar.activation(
                        znew[:, mo, :], tmp[:], RELU,
                        bias=bzu_t[:, i, mo:mo + 1])
                for mo in range(HO):
                    pn = psum.tile([P, BT], F32, name="pn", tag="ps")
                    for ko in range(HO):
                        mm(pn, wm[:, 4 * li + 3, ko, mo * P:(mo + 1) * P],
                           u[:, ko, :], ko == 0, ko == HO - 1)
                    nc.scalar.activation(
                        unew[:, mo, :], pn[:], RELU, bias=bu_t[:, i, mo:mo + 1])
                return znew, unew

            def emit_l3a(z, u):
                """L3 srl/zz (dense Wzuu[2] matmuls)."""
                srl = spool.tile([P, HO, BT], MDT, name="srl3", tag="srl", bufs=2)
                zz = spool.tile([P, HO, BT], MDT, name="zz3", tag="zz", bufs=2)
                for mo in range(HO):
                    sp = psum.tile([P, BT], F32, name="sp3", tag="ps")
                    for ko in range(HO):
                        mm(sp, wm[:, 8, ko, mo * P:(mo + 1) * P],
                           u[:, ko, :], ko == 0, ko == HO - 1)
                    nc.scalar.activation(
                        srl[:, mo, :], sp[:], RELU, bias=bzuu_t[:, 2, mo:mo + 1])
                    nc.vector.tensor_mul(zz[:, mo, :], z[:, mo, :], srl[:, mo, :])
                return zz

            def emit_l3b(t, zz, u, yt):
                """L3 final: packed (Wyuu[2], Wzu_last) + Wzzu_last + output."""
                pyu = psum.tile([P, BT], F32, name="pyu", tag="ps")
                for ko in range(HO):
                    mm(pyu, wl[:, ko, 0, :], u[:, ko, :], ko == 0, ko == HO - 1)
                pzz = psum.tile([P, BT], F32, name="pzz", tag="ps")
                for ko in range(HO):
                    mm(pzz, wl[:, ko, 1, :], zz[:, ko, :], ko == 0, ko == HO - 1)
                yy3 = spool.tile([1, BT], F32, name="yy3", tag="yy3", bufs=2)
                nc.vector.scalar_tensor_tensor(
                    yy3[:], pyu[0:1, :], byuu_t[0:1, 3:4], yt[:],
                    op0=ADD, op1=MULT)
                ta = spool.tile([1, BT], F32, name="ta", tag="ta", bufs=2)
                nc.vector.scalar_tensor_tensor(
                    ta[:], yy3[:], sc_t[0:1, 0:1], pzz[0:1, :],
                    op0=MULT, op1=ADD)
                ot = spool.tile([1, BT], F32, name="ot", tag="ot", bufs=2)
                nc.vector.scalar_tensor_tensor(
                    ot[:], pyu[32:33, :], sc_t[0:1, 1:2], ta[:], op0=ADD, op1=ADD)
                nc.sync.dma_start(out[:, t * BT:(t + 1) * BT], ot[:])

            # --- software-pipelined tile loop --------------------------------
            import contextlib
            rep_ctx = (
                tc.For_i(0, reps, 1, hint_engines=tuple(nc.engines))
                if reps > 1 else contextlib.nullcontext()
            )
            with rep_ctx:
                state = emit_l0(0, ct0, yt0)
                for t in range(NT):
                    z, u, y_b, yt = state
                    z, u = emit_mid(0, z, u, y_b)
                    z, u = emit_mid(1, z, u, y_b)
                    zz3 = emit_l3a(z, u)
                    # L0 of the next tile goes here: its cheap 2-instr PSUM
                    # groups drain (DVE/ACT) while the PE fills L3b, and the
                    # PE never sits in a long low-K stretch.
                    if t + 1 < NT:
                        state = emit_l0(t + 1)
                    emit_l3b(t, zz3, u, yt)

    nc.compile()
    return nc


def _preprocess(inputs, np_wdt=np.float16):
    """Fold scales into weights, build the replicated/transposed tensors."""
    f = lambda k: np.asarray(inputs[k], dtype=np.float32)
    p10 = float(10.0 * np.float32(inputs["p"]))
    pu10 = float(10.0 * np.float32(inputs["pu"]))
    pzu10 = float(10.0 * np.float32(inputs["pzu"]))

    Wzuu, Wzzu, Wzu, Wu = f("Wzuu"), f("Wzzu"), f("Wzu"), f("Wu")
    # s-order: li-major (Wzuu,Wzzu,Wzu,Wu) for li=0,1 then Wzuu[2]
    ws = [Wzuu[0] * pzu10, Wzzu[0] * p10, Wzu[0] * p10, Wu[0] * pu10,
          Wzuu[1] * pzu10, Wzzu[1] * p10, Wzu[1] * p10, Wu[1] * pu10,
          Wzuu[2] * pzu10]
    wmain = np.stack(ws, axis=0)                        # [9, 512, 512]
    # [9, K=512, M=512] -> [p, s, ko, m]
    wmain_r = np.ascontiguousarray(
        wmain.reshape(9, HO, P, H).transpose(2, 0, 1, 3)).astype(np_wdt)

    # w0: rows = 32 copies of (u0,u1,u2,y); weights / 32
    Wzu0, Wu0, Wyuu0 = f("Wzu0"), f("Wu0"), f("Wyuu0")
    w0blk = np.zeros((D + 1, 2 * H + 2 * P), np.float32)
    w0blk[0:D, 0:H] = Wzu0 * p10
    w0blk[0:D, H:2 * H] = Wu0 * pu10
    w0blk[0:D, 2 * H:2 * H + P] = np.repeat(Wyuu0, P, axis=1)
    w0blk[D, 2 * H + P:2 * H + 2 * P] = 1.0
    w0 = np.tile(w0blk / REP, (REP, 1)).astype(np_wdt)  # [128, 1280]

    Wyuu = f("Wyuu")  # [L, H, 1]
    wyb = np.zeros((P, 2, HO, P), np.float32)
    for li in range(2):
        col = Wyuu[li, :, 0].reshape(HO, P)             # [ko, p]
        wyb[:, li, :, :] = col.T[:, :, None]            # replicate to M=128
    wyb = wyb.astype(np_wdt)

    wl = np.zeros((P, HO, 2, P), np.float32)
    wl[:, :, 0, 0] = Wyuu[2, :, 0].reshape(HO, P).T
    wl[:, :, 0, 32] = f("Wzu_last")[:, 0].reshape(HO, P).T
    wl[:, :, 1, 0] = f("Wzzu_last")[:, 0].reshape(HO, P).T
    wl = wl.astype(np_wdt)

    Wzyu = f("Wzyu")  # [L, 1, H]
    wzyu_r = np.ascontiguousarray(
        (Wzyu[:, 0, :] * p10).reshape(L, HO, P).transpose(2, 0, 1)
    ).astype(np.float32)                                # [p, i, mo]

    def per_part(b, scale):                             # [L?, H] -> [p, l, mo]
        return np.ascontiguousarray(
            (b * scale).reshape(-1, HO, P).transpose(2, 0, 1)).astype(np.float32)

    bzuu = per_part(f("bzuu"), pzu10)                   # [128, 3, 4]
    bu = per_part(np.stack([f("bu0"), f("bu")[0], f("bu")[1]]), pu10)
    bzu = per_part(np.stack([f("bzu0"), f("bzu")[0], f("bzu")[1]]), p10)
    byuu = np.concatenate([f("byuu0"), f("byuu")[:, 0]])  # [4]
    byuu_b = np.tile(byuu[None, :], (P, 1)).astype(np.float32)
    sc = np.array([[f("Wzyu_last")[0, 0], f("bzu_last")[0]]], np.float32)

    return dict(
        w0_d=w0, wmain=wmain_r, wyb_d=wyb, wl_d=wl, wzyu_d=wzyu_r,
        bzuu_d=bzuu, bu_d=bu, bzu_d=bzu, byuu_d=byuu_b, sc_d=sc,
    )


MM_DTYPES = {
    "fp16": (mybir.dt.float16, np.float16),
    "bf16": (mybir.dt.bfloat16, None),  # ml_dtypes.bfloat16 filled lazily
}


def _run(inputs, trace=False, reps=1, mm="fp16", **kw):
    mdt, np_wdt = MM_DTYPES[mm]
    if np_wdt is None:
        import ml_dtypes
        np_wdt = ml_dtypes.bfloat16
    key = (reps, mm)
    if key not in _CACHE:
        _CACHE[key] = _build(reps, mm_dt=mdt)
    nc = _CACHE[key]
    shared = _preprocess(inputs, np_wdt)
    coords = np.asarray(inputs["coords"], dtype=np.float32)
    c4T = coords.T                                        # [4, NFULL]
    ctT = np.tile(c4T, (REP, 1)).astype(np_wdt)           # [128, NFULL]
    ytT = np.ascontiguousarray(c4T[D:D + 1, :])           # [1, NFULL]
    in_maps = [
        {**shared,
         "ct_d": np.ascontiguousarray(ctT[:, c * NC_B:(c + 1) * NC_B]),
         "yt_d": np.ascontiguousarray(ytT[:, c * NC_B:(c + 1) * NC_B])}
        for c in range(NCORES)
    ]
    res = run_bass_kernel_spmd(nc, in_maps, core_ids=list(range(NCORES)),
                               trace=trace, **kw)
    full = np.concatenate(
        [res.results[c]["out"].reshape(NC_B, 1) for c in range(NCORES)], axis=0)
    return full, res


def kernel(**inputs) -> np.ndarray:
    out, _ = _run(inputs)
    return out


# revision 24
# speedup vs baseline: 2.5949x; 1.0364x over previous
"""Trainium2 Bass kernel for nn_FCBlock (dense MLP, 8-core data parallel).

Full (unsharded) inputs in, full output out. Internally: coords are sharded
along the batch axis across 8 NeuronCores, the (few-MB) weights are
replicated, and each core runs a fused 4-stage MLP entirely out of SBUF.

Math (per point, D=3, H=512, L=3):
  y = coords[:, 3:], u = coords[:, :3]
  z = relu(10p*(u@Wzu0 + bzu0 + (y*(u@Wyuu0+byuu0))@Wzyu[0]))
  u = relu(10pu*(u@Wu0 + bu0))
  for i in 1..3:
    zz = z * relu(10pzu*(u@Wzuu[i-1] + bzuu[i-1]))
    yy = y * (u@Wyuu[i-1] + byuu[i-1])
    t  = zz@Wzzu[i-1] + u@Wzu[i-1] + bzu[i-1] + yy@Wzyu[i]     (i<3)
    t  = zz@Wzzu_last + u@Wzu_last + bzu_last + yy@Wzyu_last   (i=3 -> output)
    z, u = relu(10p*t), relu(10pu*(u@Wu[i-1] + bu[i-1]))

All 10p/10pu/10pzu scales are folded into weights/biases on the host.

Perf-critical layout decisions (from trace analysis of the v0 kernel):
 * Every matmul streams K>=128 active rows so the PE HAM never sees "low
   activity" and re-throttles to 1.2 GHz: the D=3 input matmuls are fed a
   [128, BT] input built from 32 stacked copies of coords^T (weights /32),
   and the K=1 rank-1 (yy @ Wzyu) matmuls are eliminated entirely.
 * yy is produced BROADCAST across partitions by an M=128-replicated
   stationary weight (same PE cost as M=1), and the rank-1 t3 term is
   applied on DVE via scalar_tensor_tensor against PSUM.
 * y itself is broadcast by a matmul against the 32-replicated input
   (selecting the y rows), avoiding any gpsimd/DMA broadcast.
 * Weights are pre-transposed on the host so DMAs are contiguous, and
   ordered/split so layer-0/1 weights land before they are needed.
 * Epilogues are split across ACT (srl/unew relu+bias), DVE (all PSUM
   drains: rank-1 adds, yy, L0 u), GPSIMD (SBUF->SBUF relu+bias for
   z/znew) so no single engine gates the PE.
"""

import sys

sys.path.insert(0, "/opt/trn_rl_repo")

import numpy as np

import concourse.bacc as bacc
import concourse.mybir as mybir
from concourse.tile import TileContext
from concourse.bass_utils import run_bass_kernel_spmd

NCORES = 8
NFULL = 65536
NC_B = NFULL // NCORES  # 8192 points per core
H = 512
L = 3
D = 3
P = 128
HO = H // P  # 4 partition-tiles per hidden vector
BT = 1024  # batch tile (epilogue op width)
MW = 512   # matmul moving-slice width (PSUM bank limit)
NH = BT // MW
NT = NC_B // BT
REP = P // (D + 1)  # 32 stacked copies of [u0,u1,u2,y] rows

F32 = mybir.dt.float32
RELU = mybir.ActivationFunctionType.Relu
COPY = mybir.ActivationFunctionType.Copy
ADD = mybir.AluOpType.add
MULT = mybir.AluOpType.mult
MAX = mybir.AluOpType.max

_CACHE = {}


def _build(reps=1, mm_dt=mybir.dt.float16):
    nc = bacc.Bacc(trn_type="TRN2", name="fcblock")

    MDT = mm_dt
    ct_d = nc.dram_tensor("ct_d", [P, NC_B], MDT, kind="ExternalInput")
    yt_d = nc.dram_tensor("yt_d", [1, NC_B], F32, kind="ExternalInput")
    # w0: [128, 1280]: cols 0:512 z-block, 512:1024 u-block,
    #     1024:1152 yy0-block (M=128 replicated), 1152:1280 y-broadcast block
    w0_d = nc.dram_tensor("w0_d", [P, 2 * H + 2 * P], MDT, kind="ExternalInput")
    # wmain: [128, 9, 4, 512]; s = 4*li+j, j: 0=Wzuu 1=Wzzu 2=Wzu 3=Wu (li=0,1);
    #        s=8: Wzuu[2]
    wmain = nc.dram_tensor("wmain", [P, 9, HO, H], MDT, kind="ExternalInput")
    # wyb: [128, 2, 4, 128] — Wyuu[li] column replicated to M=128, per ko
    wyb_d = nc.dram_tensor("wyb_d", [P, 2, HO, P], MDT, kind="ExternalInput")
    # wl: [128, 4, 2, 128] — two M=128-padded stationary blocks (full-width so
    # the HAM never sees a low-activity stretch): block 0 has Wyuu[2] in col 0
    # and Wzu_last in col 32 (32-aligned for DVE reads), block 1 has
    # Wzzu_last in col 0; all other columns are zero.
    wl_d = nc.dram_tensor("wl_d", [P, HO, 2, P], MDT, kind="ExternalInput")
    wzyu_d = nc.dram_tensor("wzyu_d", [P, L, HO], F32, kind="ExternalInput")
    bzuu_d = nc.dram_tensor("bzuu_d", [P, L, HO], F32, kind="ExternalInput")
    bu_d = nc.dram_tensor("bu_d", [P, L, HO], F32, kind="ExternalInput")
    bzu_d = nc.dram_tensor("bzu_d", [P, L, HO], F32, kind="ExternalInput")
    byuu_d = nc.dram_tensor("byuu_d", [P, L + 1], F32, kind="ExternalInput")
    sc_d = nc.dram_tensor("sc_d", [1, 2], F32, kind="ExternalInput")  # wzyul,bzul
    out = nc.dram_tensor("out", [1, NC_B], F32, kind="ExternalOutput")

    with TileContext(nc) as tc:
        with (
            tc.tile_pool(name="wpool", bufs=1) as wpool,
            tc.tile_pool(name="spool", bufs=2) as spool,
            tc.tile_pool(name="psum", bufs=4, space="PSUM") as psum,
        ):
            # --- tile-0 inputs first: the DMA rings drain in issue order, so
            # these must not queue behind the ~5MB of weights ----------------
            ct0 = spool.tile([P, BT], MDT, name="ct", tag="ct", bufs=3)
            nc.sync.dma_start(ct0[:], ct_d[:, 0:BT])
            yt0 = spool.tile([1, BT], F32, name="yt", tag="yt", bufs=3)
            nc.sync.dma_start(yt0[:], yt_d[:, 0:BT])

            # --- resident weights/biases (small + first-needed first) -------
            w0 = wpool.tile([P, 2 * H + 2 * P], MDT, name="w0")
            nc.sync.dma_start(w0[:], w0_d[:, :])
            wzyu_t = wpool.tile([P, L, HO], F32, name="wzyu_t")
            nc.sync.dma_start(wzyu_t[:], wzyu_d[:, :, :])
            bzuu_t = wpool.tile([P, L, HO], F32, name="bzuu_t")
            nc.sync.dma_start(bzuu_t[:], bzuu_d[:, :, :])
            bu_t = wpool.tile([P, L, HO], F32, name="bu_t")
            nc.sync.dma_start(bu_t[:], bu_d[:, :, :])
            bzu_t = wpool.tile([P, L, HO], F32, name="bzu_t")
            nc.sync.dma_start(bzu_t[:], bzu_d[:, :, :])
            byuu_t = wpool.tile([P, L + 1], F32, name="byuu_t")
            nc.sync.dma_start(byuu_t[:], byuu_d[:, :])
            sc_t = wpool.tile([1, 2], F32, name="sc_t")
            nc.sync.dma_start(sc_t[:], sc_d[:, :])
            wyb = wpool.tile([P, 2, HO, P], MDT, name="wyb")
            nc.sync.dma_start(wyb[:], wyb_d[:, :, :, :])
            wl = wpool.tile([P, HO, 2, P], MDT, name="wl")
            nc.sync.dma_start(wl[:], wl_d[:, :, :])
            wm = wpool.tile([P, 9, HO, H], MDT, name="wm")
            nc.sync.dma_start(wm[:, 0:1], wmain[:, 0:1])  # Wzuu0 (first use)
            nc.sync.dma_start(wm[:, 1:4], wmain[:, 1:4])  # rest of layer 1
            nc.sync.dma_start(wm[:, 4:8], wmain[:, 4:8])  # layer-2 weights
            nc.sync.dma_start(wm[:, 8:9], wmain[:, 8:9])  # layer-3 Wzuu

            def mm(pt, lhsT, rhs, start, stop):
                for h in range(NH):
                    nc.tensor.matmul(
                        pt[:, h * MW:(h + 1) * MW], lhsT,
                        rhs[:, h * MW:(h + 1) * MW], start=start, stop=stop)

            def emit_l0(t, ct=None, yt=None):
                """L0 for tile t as a list of thunks (one PSUM group each).

                Returns (state, thunks); running all thunks in order emits
                the full layer. The thunks are designed to be sprinkled
                between mid-layer groups of the previous tile so their
                cheap 2-instruction PSUM groups never outrun the ACT/DVE
                drains (the PE would stall on the PSUM ring otherwise).
                """
                if ct is None:
                    ct = spool.tile([P, BT], MDT, name="ct", tag="ct", bufs=3)
                    nc.sync.dma_start(ct[:], ct_d[:, t * BT:(t + 1) * BT])
                    yt = spool.tile([1, BT], F32, name="yt", tag="yt", bufs=3)
                    nc.sync.dma_start(yt[:], yt_d[:, t * BT:(t + 1) * BT])

                y_b = spool.tile([P, BT], MDT, name="y_b", tag="y_b", bufs=2)
                yy0 = spool.tile([P, BT], MDT, name="yy0", tag="yy0", bufs=2)
                z = spool.tile([P, HO, BT], MDT, name="z", tag="z", bufs=3)
                u = spool.tile([P, HO, BT], MDT, name="u", tag="u", bufs=3)

                def grp_q():
                    # q_ps = u@Wyuu0 broadcast to all 128 partitions
                    q_ps = psum.tile([P, BT], F32, name="q_ps", tag="ps")
                    mm(q_ps, w0[:, 2 * H:2 * H + P], ct, True, True)
                    # yy0 = (u@Wyuu0 + byuu0) * y  (y from the SBUF copy)
                    nc.vector.scalar_tensor_tensor(
                        yy0[:], q_ps[:], byuu_t[:, 0:1], y_b[:],
                        op0=ADD, op1=MULT)

                def grp_yb():
                    # yb_ps = y broadcast to all 128 partitions
                    yb_ps = psum.tile([P, BT], F32, name="yb_ps", tag="ps")
                    mm(yb_ps, w0[:, 2 * H + P:2 * H + 2 * P], ct, True, True)
                    nc.scalar.activation(y_b[:], yb_ps[:], COPY)

                def grp_u(mo):
                    up = psum.tile([P, BT], F32, name="up", tag="ps")
                    mm(up, w0[:, H + mo * P:H + (mo + 1) * P], ct, True, True)
                    nc.scalar.activation(
                        u[:, mo, :], up[:], RELU, bias=bu_t[:, 0, mo:mo + 1])

                def grp_z(mo):
                    zp = psum.tile([P, BT], F32, name="zp", tag="ps")
                    mm(zp, w0[:, mo * P:(mo + 1) * P], ct, True, True)
                    tmp = spool.tile([P, BT], MDT, name="tmp", tag="tmp", bufs=4)
                    nc.vector.scalar_tensor_tensor(
                        tmp[:], yy0[:], wzyu_t[:, 0, mo:mo + 1], zp[:],
                        op0=MULT, op1=ADD)
                    nc.scalar.activation(
                        z[:, mo, :], tmp[:], RELU, bias=bzu_t[:, 0, mo:mo + 1])

                thunks = ([grp_yb, grp_q]
                          + [lambda mo=mo: grp_u(mo) for mo in range(HO)]
                          + [lambda mo=mo: grp_z(mo) for mo in range(HO)])
                return (z, u, y_b, yt), thunks

            def emit_mid(li, z, u, y_b, fillers=()):
                """Layers 1..2 (li=0,1): returns (znew, unew).

                fillers: thunks (next tile's L0 groups) interleaved one per
                mid-layer PSUM group so their drains overlap dense work.
                """
                fillers = list(fillers)

                def fill():
                    if fillers:
                        fillers.pop(0)()

                i = li + 1
                srl = spool.tile([P, HO, BT], MDT, name="srl", tag="srl", bufs=2)
                zz = spool.tile([P, HO, BT], MDT, name="zz", tag="zz", bufs=2)
                for mo in range(HO):
                    sp = psum.tile([P, BT], F32, name="sp", tag="ps")
                    for ko in range(HO):
                        mm(sp, wm[:, 4 * li + 0, ko, mo * P:(mo + 1) * P],
                           u[:, ko, :], ko == 0, ko == HO - 1)
                    nc.scalar.activation(
                        srl[:, mo, :], sp[:], RELU, bias=bzuu_t[:, li, mo:mo + 1])
                    nc.vector.tensor_mul(zz[:, mo, :], z[:, mo, :], srl[:, mo, :])
                    fill()

                # yy broadcast: M=128-replicated Wyuu[li] column
                pyp = psum.tile([P, BT], F32, name="pyp", tag="ps")
                for ko in range(HO):
                    mm(pyp, wyb[:, li, ko, :], u[:, ko, :], ko == 0, ko == HO - 1)
                yy_b = spool.tile([P, BT], MDT, name="yy_b", tag="yy_b", bufs=2)
                nc.vector.scalar_tensor_tensor(
                    yy_b[:], pyp[:], byuu_t[:, i:i + 1], y_b[:],
                    op0=ADD, op1=MULT)

                znew = spool.tile([P, HO, BT], MDT, name="znew", tag="z", bufs=3)
                unew = spool.tile([P, HO, BT], MDT, name="unew", tag="u", bufs=3)
                for mo in range(HO):
                    tp = psum.tile([P, BT], F32, name="tp", tag="ps")
                    for ko in range(HO):
                        mm(tp, wm[:, 4 * li + 1, ko, mo * P:(mo + 1) * P],
                           zz[:, ko, :], ko == 0, False)
                    for ko in range(HO):
                        mm(tp, wm[:, 4 * li + 2, ko, mo * P:(mo + 1) * P],
                           u[:, ko, :], False, ko == HO - 1)
                    tmp = spool.tile([P, BT], MDT, name="tmp", tag="tmp", bufs=4)
                    nc.vector.scalar_tensor_tensor(
                        tmp[:], yy_b[:], wzyu_t[:, i, mo:mo + 1], tp[:],
                        op0=MULT, op1=ADD)
                    nc.scalar.activation(
                        znew[:, mo, :], tmp[:], RELU,
                        bias=bzu_t[:, i, mo:mo + 1])
                    fill()
                for mo in range(HO):
                    pn = psum.tile([P, BT], F32, name="pn", tag="ps")
                    for ko in range(HO):
                        mm(pn, wm[:, 4 * li + 3, ko, mo * P:(mo + 1) * P],
                           u[:, ko, :], ko == 0, ko == HO - 1)
                    nc.scalar.activation(
                        unew[:, mo, :], pn[:], RELU, bias=bu_t[:, i, mo:mo + 1])
                    fill()
                while fillers:
                    fillers.pop(0)()
                return znew, unew

            def emit_l3a(z, u):
                """L3 srl/zz (dense Wzuu[2] matmuls)."""
                srl = spool.tile([P, HO, BT], MDT, name="srl3", tag="srl", bufs=2)
                zz = spool.tile([P, HO, BT], MDT, name="zz3", tag="zz", bufs=2)
                for mo in range(HO):
                    sp = psum.tile([P, BT], F32, name="sp3", tag="ps")
                    for ko in range(HO):
                        mm(sp, wm[:, 8, ko, mo * P:(mo + 1) * P],
                           u[:, ko, :], ko == 0, ko == HO - 1)
                    nc.scalar.activation(
                        srl[:, mo, :], sp[:], RELU, bias=bzuu_t[:, 2, mo:mo + 1])
                    nc.vector.tensor_mul(zz[:, mo, :], z[:, mo, :], srl[:, mo, :])
                return zz

            def emit_l3b(t, zz, u, yt):
                """L3 final: packed (Wyuu[2], Wzu_last) + Wzzu_last + output."""
                pyu = psum.tile([P, BT], F32, name="pyu", tag="ps")
                for ko in range(HO):
                    mm(pyu, wl[:, ko, 0, :], u[:, ko, :], ko == 0, ko == HO - 1)
                pzz = psum.tile([P, BT], F32, name="pzz", tag="ps")
                for ko in range(HO):
                    mm(pzz, wl[:, ko, 1, :], zz[:, ko, :], ko == 0, ko == HO - 1)
                yy3 = spool.tile([1, BT], F32, name="yy3", tag="yy3", bufs=2)
                nc.vector.scalar_tensor_tensor(
                    yy3[:], pyu[0:1, :], byuu_t[0:1, 3:4], yt[:],
                    op0=ADD, op1=MULT)
                ta = spool.tile([1, BT], F32, name="ta", tag="ta", bufs=2)
                nc.vector.scalar_tensor_tensor(
                    ta[:], yy3[:], sc_t[0:1, 0:1], pzz[0:1, :],
                    op0=MULT, op1=ADD)
                ot = spool.tile([1, BT], F32, name="ot", tag="ot", bufs=2)
                nc.vector.scalar_tensor_tensor(
                    ot[:], pyu[32:33, :], sc_t[0:1, 1:2], ta[:], op0=ADD, op1=ADD)
                nc.sync.dma_start(out[:, t * BT:(t + 1) * BT], ot[:])

            # --- software-pipelined tile loop --------------------------------
            import contextlib
            rep_ctx = (
                tc.For_i(0, reps, 1, hint_engines=tuple(nc.engines))
                if reps > 1 else contextlib.nullcontext()
            )
            with rep_ctx:
                state, thunks = emit_l0(0, ct0, yt0)
                for th in thunks:  # prologue: no previous tile to hide in
                    th()
                for t in range(NT):
                    z, u, y_b, yt = state
                    z, u = emit_mid(0, z, u, y_b)
                    if t + 1 < NT:
                        state, thunks = emit_l0(t + 1)
                    else:
                        thunks = ()
                    z, u = emit_mid(1, z, u, y_b, fillers=thunks)
                    zz3 = emit_l3a(z, u)
                    emit_l3b(t, zz3, u, yt)

    nc.compile()
    return nc


def _preprocess(inputs, np_wdt=np.float16):
    """Fold scales into weights, build the replicated/transposed tensors."""
    f = lambda k: np.asarray(inputs[k], dtype=np.float32)
    p10 = float(10.0 * np.float32(inputs["p"]))
    pu10 = float(10.0 * np.float32(inputs["pu"]))
    pzu10 = float(10.0 * np.float32(inputs["pzu"]))

    Wzuu, Wzzu, Wzu, Wu = f("Wzuu"), f("Wzzu"), f("Wzu"), f("Wu")
    # s-order: li-major (Wzuu,Wzzu,Wzu,Wu) for li=0,1 then Wzuu[2]
    ws = [Wzuu[0] * pzu10, Wzzu[0] * p10, Wzu[0] * p10, Wu[0] * pu10,
          Wzuu[1] * pzu10, Wzzu[1] * p10, Wzu[1] * p10, Wu[1] * pu10,
          Wzuu[2] * pzu10]
    wmain = np.stack(ws, axis=0)                        # [9, 512, 512]
    # [9, K=512, M=512] -> [p, s, ko, m]
    wmain_r = np.ascontiguousarray(
        wmain.reshape(9, HO, P, H).transpose(2, 0, 1, 3)).astype(np_wdt)

    # w0: rows = 32 copies of (u0,u1,u2,y); weights / 32
    Wzu0, Wu0, Wyuu0 = f("Wzu0"), f("Wu0"), f("Wyuu0")
    w0blk = np.zeros((D + 1, 2 * H + 2 * P), np.float32)
    w0blk[0:D, 0:H] = Wzu0 * p10
    w0blk[0:D, H:2 * H] = Wu0 * pu10
    w0blk[0:D, 2 * H:2 * H + P] = np.repeat(Wyuu0, P, axis=1)
    w0blk[D, 2 * H + P:2 * H + 2 * P] = 1.0
    w0 = np.tile(w0blk / REP, (REP, 1)).astype(np_wdt)  # [128, 1280]

    Wyuu = f("Wyuu")  # [L, H, 1]
    wyb = np.zeros((P, 2, HO, P), np.float32)
    for li in range(2):
        col = Wyuu[li, :, 0].reshape(HO, P)             # [ko, p]
        wyb[:, li, :, :] = col.T[:, :, None]            # replicate to M=128
    wyb = wyb.astype(np_wdt)

    wl = np.zeros((P, HO, 2, P), np.float32)
    wl[:, :, 0, 0] = Wyuu[2, :, 0].reshape(HO, P).T
    wl[:, :, 0, 32] = f("Wzu_last")[:, 0].reshape(HO, P).T
    wl[:, :, 1, 0] = f("Wzzu_last")[:, 0].reshape(HO, P).T
    wl = wl.astype(np_wdt)

    Wzyu = f("Wzyu")  # [L, 1, H]
    wzyu_r = np.ascontiguousarray(
        (Wzyu[:, 0, :] * p10).reshape(L, HO, P).transpose(2, 0, 1)
    ).astype(np.float32)                                # [p, i, mo]

    def per_part(b, scale):                             # [L?, H] -> [p, l, mo]
        return np.ascontiguousarray(
            (b * scale).reshape(-1, HO, P).transpose(2, 0, 1)).astype(np.float32)

    bzuu = per_part(f("bzuu"), pzu10)                   # [128, 3, 4]
    bu = per_part(np.stack([f("bu0"), f("bu")[0], f("bu")[1]]), pu10)
    bzu = per_part(np.stack([f("bzu0"), f("bzu")[0], f("bzu")[1]]), p10)
    byuu = np.concatenate([f("byuu0"), f("byuu")[:, 0]])  # [4]
    byuu_b = np.tile(byuu[None, :], (P, 1)).astype(np.float32)
    sc = np.array([[f("Wzyu_last")[0, 0], f("bzu_last")[0]]], np.float32)

    return dict(
        w0_d=w0, wmain=wmain_r, wyb_d=wyb, wl_d=wl, wzyu_d=wzyu_r,
        bzuu_d=bzuu, bu_d=bu, bzu_d=bzu, byuu_d=byuu_b, sc_d=sc,
    )


MM_DTYPES = {
    "fp16": (mybir.dt.float16, np.float16),
    "bf16": (mybir.dt.bfloat16, None),  # ml_dtypes.bfloat16 filled lazily
}


def _run(inputs, trace=False, reps=1, mm="fp16", **kw):
    mdt, np_wdt = MM_DTYPES[mm]
    if np_wdt is None:
        import ml_dtypes
        np_wdt = ml_dtypes.bfloat16
    key = (reps, mm)
    if key not in _CACHE:
        _CACHE[key] = _build(reps, mm_dt=mdt)
    nc = _CACHE[key]
    shared = _preprocess(inputs, np_wdt)
    coords = np.asarray(inputs["coords"], dtype=np.float32)
    c4T = coords.T                                        # [4, NFULL]
    ctT = np.tile(c4T, (REP, 1)).astype(np_wdt)           # [128, NFULL]
    ytT = np.ascontiguousarray(c4T[D:D + 1, :])           # [1, NFULL]
    in_maps = [
        {**shared,
         "ct_d": np.ascontiguousarray(ctT[:, c * NC_B:(c + 1) * NC_B]),
         "yt_d": np.ascontiguousarray(ytT[:, c * NC_B:(c + 1) * NC_B])}
        for c in range(NCORES)
    ]
    res = run_bass_kernel_spmd(nc, in_maps, core_ids=list(range(NCORES)),
                               trace=trace, **kw)
    full = np.concatenate(
        [res.results[c]["out"].reshape(NC_B, 1) for c in range(NCORES)], axis=0)
    return full, res


def kernel(**inputs) -> np.ndarray:
    out, _ = _run(inputs)
    return out
